# revision 1
# baseline (speedup 1.0000x reference)
"""Trainium2 Bass kernel: offset-attention transformer block (nn_OA_2b).

Computation (per batch b):
    x_q = (Wq @ q)^T            [N, 64]
    x_k = Wk @ x                [64, N]
    x_v = Wv @ q + bv           [256, N]
    E   = x_q @ x_k             [N, N]
    A   = softmax_rows(E)
    A   = A / (1e-9 + colsum(A))
    x_r = x_v @ A               [256, N]
    t   = Wt @ (x - x_r) + bt
    out = relu(batchnorm(t))    (batch stats over all B and N)

Sharding: data-parallel over batch, one batch per NeuronCore (B=8, 8 cores).
The BatchNorm statistics couple the batches -> tiny AllReduce of per-channel
(sum, sumsq) at the end.

Kernel strategy per core (all matmuls float32r: full PE speed, FP22 mult,
fp32 accumulate):
  - Projections via PE; x_v kept transposed [N, 257] with a ones column
    (column 256) so the attention-output matmul also produces colsum.
  - Pass 1 (row-major): energy strips E[n128, 2048] in PSUM, ACT Exp with
    fused accum_out gives softmax row-sums (no max subtraction needed:
    logits are O(40) < 88, exp fits fp32).
  - Row normalization folded as A = exp(e - ln(rowsum)) via per-partition
    ACT bias; 1/rowsum also folded into x_vT.
  - Pass 2 (m-major): recompute energy tile, exp, accumulate
    x_rT[m128, 257] = sum_n A^T-chunk . x_vT-chunk in PSUM over all n.
    Column 256 = colsum.  Scale by 1/(1e-9+colsum), PE-transpose 128x128
    blocks, and subtract in place into x (producing d = x - x_r).
  - t = WtT^T @ d (+bt via ACT bias), per-channel sum/sumsq, AllReduce,
    then out = relu(a*t + b) with folded BN affine.
"""

import numpy as np

import concourse.bass as bass
import concourse.bacc as bacc
import concourse.mybir as mybir
import concourse.tile as tile
from concourse.masks import make_identity

F32 = mybir.dt.float32
F32R = mybir.dt.float32r
BF16 = mybir.dt.bfloat16
AF = mybir.ActivationFunctionType
ALU = mybir.AluOpType
AX = mybir.AxisListType

B, CM, DX, N = 8, 256, 128, 4096
CM4 = CM // 4            # 64
NCH = N // 128           # 32 chunks of 128 points
NMG = N // 512           # 8 m-groups of 512
BN_EPS = 1e-5
ESHIFT = 40.0          # exp(e - ESHIFT): keeps rowsums well inside Ln's range
NCORES = 8
CMP = CM + 2           # augmented width: 256 ch + colsum-ones col + pad (even)


def _r(ap):
    """View an fp32 AP as float32r so the PE runs at full (1 cyc/row) rate."""
    return ap.bitcast(F32R)


def build_nc():
    nc = bacc.Bacc(None, num_devices=NCORES)

    dq = nc.dram_tensor("q", [DX, N], F32R, kind="ExternalInput")
    dx = nc.dram_tensor("x", [CM, N], F32R, kind="ExternalInput")
    dwqT = nc.dram_tensor("wqT", [DX, CM4], F32R, kind="ExternalInput")
    dwkT = nc.dram_tensor("wkT", [CM, CM4], F32R, kind="ExternalInput")
    dwvT = nc.dram_tensor("wvT", [DX, CMP], F32R, kind="ExternalInput")
    dwtT = nc.dram_tensor("wtT", [CM, CM], F32R, kind="ExternalInput")
    dbvb = nc.dram_tensor("bvb", [128, CMP], F32, kind="ExternalInput")
    dbt = nc.dram_tensor("btc", [128, 2], F32, kind="ExternalInput")
    dga = nc.dram_tensor("gac", [128, 2], F32, kind="ExternalInput")
    dbe = nc.dram_tensor("bec", [128, 2], F32, kind="ExternalInput")
    dout = nc.dram_tensor("out", [CM, N], F32, kind="ExternalOutput")

    with tile.TileContext(nc) as tc:
        _build(nc, tc, dq, dx, dwqT, dwkT, dwvT, dwtT, dbvb, dbt, dga, dbe, dout)
    nc.compile()
    return nc


def _build(nc, tc, dq, dx, dwqT, dwkT, dwvT, dwtT, dbvb, dbt, dga, dbe, dout):
    from contextlib import ExitStack

    ctx = ExitStack()
    with ctx:
        consts = ctx.enter_context(tc.tile_pool(name="consts", bufs=1))
        pbig = ctx.enter_context(tc.tile_pool(name="pbig", bufs=1))
        small = ctx.enter_context(tc.tile_pool(name="small", bufs=4))

        # ---- constant / weight loads ----
        wqT = consts.tile([128, CM4], F32R)
        nc.sync.dma_start(wqT, dwqT[:])
        wkT = [consts.tile([128, CM4], F32R, tag=f"wkT{k}", name=f"wkT{k}") for k in range(2)]
        for k in range(2):
            nc.sync.dma_start(wkT[k], dwkT[k * 128:(k + 1) * 128, :])
        wvT = consts.tile([128, CMP], F32R)
        nc.sync.dma_start(wvT, dwvT[:])
        wtT = [consts.tile([128, CM], F32R, tag=f"wtT{k}", name=f"wtT{k}") for k in range(2)]
        for k in range(2):
            nc.sync.dma_start(wtT[k], dwtT[k * 128:(k + 1) * 128, :])
        bvb = consts.tile([128, CMP], F32)
        nc.sync.dma_start(bvb, dbvb[:])
        btc = consts.tile([128, 2], F32)
        nc.sync.dma_start(btc, dbt[:])
        gac = consts.tile([128, 2], F32)
        nc.sync.dma_start(gac, dga[:])
        bec = consts.tile([128, 2], F32)
        nc.sync.dma_start(bec, dbe[:])
        ident = consts.tile([128, 128], F32)
        make_identity(nc, ident[:])

        # ---- big persistent tensors ----
        q = pbig.tile([128, N], F32R, tag="qt")           # later reused for t0
        nc.sync.dma_start(q, dq[:])
        xs = [pbig.tile([128, N], F32R, tag=f"x{c}", name=f"xs{c}") for c in range(2)]
        for c in range(2):
            nc.sync.dma_start(xs[c], dx[c * 128:(c + 1) * 128, :])
        xqT = pbig.tile([CM4, N], F32R, tag="xqt")        # [64, 4096]; reused for t1
        xk = pbig.tile([CM4, N], F32R, tag="xk")          # [64, 4096]
        xvt = pbig.tile([128, NCH, CMP], BF16, tag="xvt")  # x_v^T chunks + ones col

        # softmax row stats
        rs_part = consts.tile([128, NCH, 2], F32)
        rs = consts.tile([128, NCH], F32)
        lnrs = consts.tile([128, NCH], F32)
        negln = consts.tile([128, NCH], F32)
        nshift = consts.tile([128, 1], F32)
        nc.vector.memset(nshift, -ESHIFT)

        # ---- projections ----
        with tc.tile_pool(name="psB", bufs=4, space="PSUM") as psB:
            # x_qT[o, n] = sum_d WqT[d, o] q[d, n]
            for mi in range(NMG):
                pt = psB.tile([CM4, 512], F32, tag="ps")
                nc.tensor.matmul(pt, lhsT=(wqT[:]), rhs=(q[:, mi * 512:(mi + 1) * 512]),
                                 start=True, stop=True)
                nc.scalar.copy(xqT[:, mi * 512:(mi + 1) * 512], pt)
            # x_k[o, m] = sum_c WkT[c, o] x[c, m]
            for mi in range(NMG):
                pt = psB.tile([CM4, 512], F32, tag="ps")
                for k in range(2):
                    nc.tensor.matmul(pt, lhsT=(wkT[k][:]),
                                     rhs=(xs[k][:, mi * 512:(mi + 1) * 512]),
                                     start=(k == 0), stop=(k == 1))
                nc.scalar.copy(xk[:, mi * 512:(mi + 1) * 512], pt)
            # x_vT[n, c] = sum_d q[d, n] WvT[d, c]  (+ bv broadcast; col 256
            # of wvT is zero and col 256 of bvb is one -> ones column of xvt)
            for ni in range(NCH):
                pt = psB.tile([128, CMP], F32, tag="ps")
                nc.tensor.matmul(pt, lhsT=(q[:, ni * 128:(ni + 1) * 128]), rhs=(wvT[:]),
                                 start=True, stop=True)
                nc.vector.tensor_add(xvt[:, ni, :], pt, bvb)

        # ---- pass 1: softmax row sums ----
        with tc.tile_pool(name="psC", bufs=2, space="PSUM") as psC, \
             tc.tile_pool(name="scrp", bufs=2) as scrp:
            for ni in range(NCH):
                lhs = (xqT[:, ni * 128:(ni + 1) * 128])
                for g in range(2):
                    pg = psC.tile([128, 2048], F32, tag="pg")
                    for s in range(4):
                        m0 = g * 2048 + s * 512
                        nc.tensor.matmul(pg[:, s * 512:(s + 1) * 512], lhsT=lhs,
                                         rhs=(xk[:, m0:m0 + 512]),
                                         start=True, stop=True)
                    scr = scrp.tile([128, 2048], BF16, tag="scr")
                    nc.scalar.activation(scr, pg, AF.Exp, bias=nshift[:],
                                         accum_out=rs_part[:, ni, g:g + 1])

        nc.vector.tensor_add(rs, rs_part[:, :, 0], rs_part[:, :, 1])
        nc.scalar.activation(lnrs, rs, AF.Ln)
        # A = exp(e - ESHIFT - ln(rowsum')) is the exact row-softmax
        nc.vector.tensor_scalar(negln, lnrs, -1.0, -ESHIFT, ALU.mult, ALU.add)

        # ---- pass 2: attention output (transposed) + colsum, fused subtract ----
        with tc.tile_pool(name="accp", bufs=4, space="PSUM") as accp, \
             tc.tile_pool(name="epp", bufs=2, space="PSUM") as epp, \
             tc.tile_pool(name="tpp", bufs=2, space="PSUM") as tpp, \
             tc.tile_pool(name="apool", bufs=3) as apool, \
             tc.tile_pool(name="xrtp", bufs=3) as xrtp:
            for g in range(NMG):
                ms = g * 512
                acc = [accp.tile([128, CMP], F32, tag="acc", name=f"acc{g}_{jj}") for jj in range(4)]
                ep_prev = epp.tile([128, 512], F32, tag="ep")
                nc.tensor.matmul(ep_prev, lhsT=(xqT[:, 0:128]),
                                 rhs=(xk[:, ms:ms + 512]), start=True, stop=True)
                for ni in range(NCH):
                    if ni + 1 < NCH:
                        ep_next = epp.tile([128, 512], F32, tag="ep")
                        nc.tensor.matmul(ep_next,
                                         lhsT=(xqT[:, (ni + 1) * 128:(ni + 2) * 128]),
                                         rhs=(xk[:, ms:ms + 512]),
                                         start=True, stop=True)
                    at = apool.tile([128, 512], BF16, tag="A")
                    nc.scalar.activation(at, ep_prev, AF.Exp,
                                         bias=negln[:, ni:ni + 1], scale=1.0)
                    for j in range(4):
                        nc.tensor.matmul(acc[j], lhsT=(at[:, j * 128:(j + 1) * 128]),
                                         rhs=(xvt[:, ni, :]),
                                         start=(ni == 0), stop=(ni == NCH - 1))
                    if ni + 1 < NCH:
                        ep_prev = ep_next
                for j in range(4):
                    m128 = g * 4 + j
                    cseps = small.tile([128, 1], F32, tag="cs")
                    nc.vector.tensor_scalar_add(cseps, acc[j][:, CM:CM + 1], 1e-9)
                    rc = small.tile([128, 1], F32, tag="rc")
                    nc.vector.reciprocal(rc, cseps)
                    xrt = xrtp.tile([128, CM], F32, tag="xrt")
                    nc.vector.tensor_scalar_mul(xrt, acc[j][:, 0:CM], rc)
                    for c in range(2):
                        tp = tpp.tile([128, 128], F32, tag="tp")
                        nc.tensor.transpose(tp, xrt[:, c * 128:(c + 1) * 128], ident[:])
                        col = m128 * 128
                        nc.vector.tensor_tensor(out=xs[c][:, col:col + 128],
                                                in0=xs[c][:, col:col + 128],
                                                in1=tp, op=ALU.subtract)

        # ---- t = WtT^T @ d + bt;  BN stats; AllReduce; relu(a*t+b) ----
        stats = consts.tile([128, 4], F32)
        gstats = consts.tile([128, 4], F32)
        t0 = pbig.tile([128, N], F32, tag="qt")
        t1 = pbig.tile([128, N], F32, tag="xqt")
        ts_ = [t0, t1]
        with tc.tile_pool(name="psE", bufs=3, space="PSUM") as psE:
            for oc in range(2):
                for mi in range(NMG):
                    pt = psE.tile([128, 512], F32, tag="t")
                    for kc in range(2):
                        nc.tensor.matmul(pt,
                                         lhsT=(wtT[kc][:, oc * 128:(oc + 1) * 128]),
                                         rhs=(xs[kc][:, mi * 512:(mi + 1) * 512]),
                                         start=(kc == 0), stop=(kc == 1))
                    nc.scalar.activation(ts_[oc][:, mi * 512:(mi + 1) * 512], pt,
                                         AF.Identity, bias=btc[:, oc:oc + 1], scale=1.0)
            for oc in range(2):
                nc.vector.tensor_reduce(out=stats[:, oc:oc + 1], in_=ts_[oc],
                                        axis=AX.X, op=ALU.add)
                sq = pbig.tile([128, N], F32, tag="xvt")
                nc.scalar.activation(sq, ts_[oc], AF.Square,
                                     accum_out=stats[:, 2 + oc:3 + oc])

        with tc.tile_pool(name="dramp", bufs=1, space="DRAM") as dramp:
            cin = dramp.tile([128, 4], F32)
            cout = dramp.tile([128, 4], F32)
            nc.sync.dma_start(cin, stats)
            nc.gpsimd.collective_compute(
                "AllReduce", ALU.add,
                replica_groups=[list(range(NCORES))],
                ins=[cin.opt()], outs=[cout.opt()])
            nc.sync.dma_start(gstats, cout)

        invn = 1.0 / float(B * N)
        mean = small.tile([128, 2], F32, tag="bn")
        ex2 = small.tile([128, 2], F32, tag="bn")
        var = small.tile([128, 2], F32, tag="bn")
        sd = small.tile([128, 2], F32, tag="bn")
        rstd = small.tile([128, 2], F32, tag="bn2")
        a_ = small.tile([128, 2], F32, tag="bn2")
        amean = small.tile([128, 2], F32, tag="bn2")
        b_ = small.tile([128, 2], F32, tag="bn2")
        nc.vector.tensor_scalar_mul(mean, gstats[:, 0:2], invn)
        nc.vector.tensor_scalar_mul(ex2, gstats[:, 2:4], invn)
        nc.vector.tensor_mul(var, mean, mean)
        nc.vector.tensor_sub(var, ex2, var)
        nc.vector.tensor_scalar_add(var, var, BN_EPS)
        nc.scalar.activation(sd, var, AF.Sqrt)
        nc.vector.reciprocal(rstd, sd)
        nc.vector.tensor_mul(a_, gac, rstd)
        nc.vector.tensor_mul(amean, a_, mean)
        nc.vector.tensor_sub(b_, bec, amean)

        for oc in range(2):
            o = pbig.tile([128, N], F32, tag=f"x{oc}", name=f"o{oc}")
            nc.scalar.activation(o, ts_[oc], AF.Relu,
                                 bias=b_[:, oc:oc + 1], scale=a_[:, oc:oc + 1])
            nc.sync.dma_start(dout[oc * 128:(oc + 1) * 128, :], o)


_CACHE = {}


def _get_nc():
    if "nc" not in _CACHE:
        _CACHE["nc"] = build_nc()
    return _CACHE["nc"]


def _make_in_maps(inputs):
    f = lambda a: np.ascontiguousarray(np.asarray(a, dtype=np.float32))
    q = f(inputs["q"])
    x = f(inputs["x"])
    shared = {
        "wqT": f(np.asarray(inputs["Wq"]).T),
        "wkT": f(np.asarray(inputs["Wk"]).T),
        "wvT": f(np.concatenate([np.asarray(inputs["Wv"]).T, np.zeros((DX, 2), np.float32)], axis=1)),
        "wtT": f(np.asarray(inputs["Wt"]).T),
        "bvb": f(np.concatenate([np.broadcast_to(np.asarray(inputs["bv"]), (128, CM)), np.ones((128, 1), np.float32), np.zeros((128, 1), np.float32)], axis=1)),
        "btc": f(np.asarray(inputs["bt"]).reshape(2, 128).T),
        "gac": f(np.asarray(inputs["gamma"]).reshape(2, 128).T),
        "bec": f(np.asarray(inputs["beta"]).reshape(2, 128).T),
    }
    return [{"q": q[b], "x": x[b], **shared} for b in range(NCORES)]


def run(inputs, trace=False):
    """Run on the 8 NeuronCores; returns (out [8,256,4096], exec_time_ns|None)."""
    from concourse.bass_utils import run_bass_kernel_spmd

    nc = _get_nc()
    in_maps = _make_in_maps(inputs)
    res = run_bass_kernel_spmd(nc, in_maps, core_ids=list(range(NCORES)),
                               trace=trace)
    out = np.stack([res.results[b]["out"] for b in range(NCORES)], axis=0)
    return out, res.exec_time_ns


def kernel(**inputs) -> np.ndarray:
    out, _ = run(inputs)
    return out

